# revision 7
# baseline (speedup 1.0000x reference)
"""Trainium2 Bass kernel for BaseSSMLayer — sequence-sharded (v2).

Computation (verified equivalent to the reference's associative_scan):
    U = xs @ w_in.T              # [L, N]
    h_t = lam * h_{t-1} + U_t    # linear recurrence over L
    Y = H @ c_out.T + xs * d_skip

Strategy: data-parallel over the sequence (L=16384 -> 2048 steps/core,
8 cores).  Each core holds the FULL weights (w 8MB + c 8MB bf16) and
computes its chunk's mm1 -> local scan (zero init) -> mm2 exactly like
the v1 tensor-parallel kernel, but with 4x less HBM traffic per core
(32MB vs 130MB): no full-xs broadcast, no full-size partial-Y output.
Less DMA energy also means less GPIO/P0 clock throttling, which set the
worst-core span in v1.

The scan's cross-chunk carry is fixed up on the HOST: each core outputs
its local end-state e_k ([2048] f32, 8KB).  The host chains
carry_k = lam^T * carry_{k-1} + e_{k-1} and adds the correction
    Y_k += c_out @ (carry_k * lam^{t+1})
via a low-rank factorization of the decay-profile matrix
Lpow[n, t] = lam_n^{t+1}  (exponential profiles are numerically
low-rank; anchor-basis least squares with closed-form Gram matrices
gives rel err ~1e-5 at r=48).  No cross-core communication on device.
"""

import numpy as np
import ml_dtypes

import concourse.tile as tile
from concourse import bacc, mybir
from concourse.bass import ts
from concourse.bass_utils import run_bass_kernel_spmd

L = 16384        # sequence length
I = 2048         # in_dim (= out dim of Y)
N = 2048         # state_dim
NCORES = 8
T = L // NCORES          # 2048 timesteps per core
TT = 512                 # time-tile (free dim per matmul / scan)
NT = T // TT             # 4 time slabs per core
KI = I // 128            # 16 contraction tiles over in_dim (mm1)
NG = N // 128            # 16 state-channel groups (scan partitions, mm2 contraction)
NO = I // 128            # 16 output blocks (mm2)
NCH = 4                  # x DMA chunking: KI split into NCH chunks of KC i-tiles
KC = KI // NCH

BF16 = mybir.dt.bfloat16
F32 = mybir.dt.float32
NP_BF16 = ml_dtypes.bfloat16


def _build_nc():
    nc = bacc.Bacc(
        "TRN2",
        target_bir_lowering=False,
        debug=False,
        num_devices=NCORES,
    )
    xt = nc.dram_tensor("xt", [NT, 128, KI * TT], BF16, kind="ExternalInput").ap()
    wt = nc.dram_tensor("wt", [I, N], BF16, kind="ExternalInput").ap()
    ct = nc.dram_tensor("ct", [N, I], BF16, kind="ExternalInput").ap()
    lamb = nc.dram_tensor("lamb", [128, NG], F32, kind="ExternalInput").ap()
    y = nc.dram_tensor("y", [NT, 128, NO * TT], BF16, kind="ExternalOutput").ap()
    e = nc.dram_tensor("e", [128, NG], F32, kind="ExternalOutput").ap()

    with tile.TileContext(nc) as tc:
        with (
            tc.tile_pool(name="const", bufs=1) as const_pool,
            tc.tile_pool(name="xin", bufs=6) as x_pool,
            tc.tile_pool(name="hb", bufs=34) as hb_pool,
            tc.tile_pool(name="yst", bufs=3) as yst_pool,
            tc.tile_pool(name="ups", bufs=4, space="PSUM") as u_psum,
            tc.tile_pool(name="yps", bufs=4, space="PSUM") as y_psum,
        ):
            # --- startup DMA orchestration: w split across BOTH queues,
            # interleaved with the slab-0 x chunks in consumption order so
            # the slab-0 mm1 rounds never starve (stalls >3.4us would also
            # re-throttle the PE clock governor). ---
            lam_sb = const_pool.tile([128, NG], F32, tag="lam")
            nc.scalar.dma_start(lam_sb[:], lamb[:])
            w_sb = [None] * KI
            xt3 = xt.rearrange("t p (j c) -> t p j c", j=NCH)

            def dma_w(i, eng):
                w = const_pool.tile([128, N], BF16, tag=f"w{i}", name=f"w{i}")
                eng.dma_start(w[:], wt[i * 128:(i + 1) * 128, :])
                w_sb[i] = w

            def w_slice(i, g):
                return w_sb[i][:, g * 128:(g + 1) * 128]

            def dma_x_chunk(t, j, eng):
                xc = x_pool.tile([128, KC * TT], BF16, tag="x", name=f"x{t}_{j}")
                eng.dma_start(xc[:], xt3[t, :, j])
                return xc

            # startup: 512KB granules, both queues, ordered by first-need time
            #   scalar: lam w0 w2 w4 w6 w8 w10 w12 w14   (then idles; c gated)
            #   sync:   x0c0 w1 w3 x0c1 w5 w7 x0c2 w9 w11 x0c3 w13 w15
            x0_chunks = [None] * NCH
            x0_chunks[0] = dma_x_chunk(0, 0, nc.sync)
            dma_w(1, nc.sync)
            dma_w(3, nc.sync)
            x0_chunks[1] = dma_x_chunk(0, 1, nc.sync)
            dma_w(5, nc.sync)
            dma_w(7, nc.sync)
            x0_chunks[2] = dma_x_chunk(0, 2, nc.sync)
            dma_w(9, nc.sync)
            dma_w(11, nc.sync)
            x0_chunks[3] = dma_x_chunk(0, 3, nc.sync)
            dma_w(13, nc.sync)
            dma_w(15, nc.sync)
            for i in (0, 2, 4, 6, 8, 10, 12, 14):
                dma_w(i, nc.scalar)
            # Pre-warm the PE clock governor while the startup DMA lands
            # (512-wide moving operand keeps the governor at 2.4 GHz); the
            # slab-0 8-wide i-major passes only need w_0/x0c0 to start, so a
            # short bridge suffices.
            warm_sb = const_pool.tile([128, 512], BF16, tag="warm")
            nc.vector.memset(warm_sb[:], 0.0)
            warm_ps = u_psum.tile([128, TT], F32, tag="u")
            for _ in range(36):
                nc.tensor.matmul(warm_ps[:], warm_sb[:, :128], warm_sb[:], start=True, stop=True)

            c_sb = [None] * NG
            gate_sb = const_pool.tile([128, NG], F32, tag="gate")

            def dma_c(g, hb):
                # c loads ride the scalar queue but are gated behind a tiny
                # ACT copy that reads slab-0 scan g's output, so their 8MB
                # never contends with the critical w/x startup DMA.
                nc.scalar.copy(gate_sb[:, g:g + 1], hb[:, :1])
                c = const_pool.tile([128, I], BF16, tag=f"c{g}", name=f"c{g}")
                nc.scalar.dma_start(c[:], ct[g * 128:(g + 1) * 128, :])
                c_sb[g] = c

            e_sb = const_pool.tile([128, NG], F32, tag="e")

            hb_prev = [None] * NG          # bf16 H tile of previous slab, per group
            hb_tiles = [None] * (NT * NG)  # bf16 H tiles pending mm2
            yo3 = y.rearrange("t p (j c) -> t p j c", j=NO // 4)
            yo2 = y.rearrange("t p (j c) -> t p j c", j=NO // 2)

            def dma_x(t):
                return [dma_x_chunk(t, j, nc.sync) for j in range(NCH)]

            def mm1_one(u_ps, x_chunks, g, i):
                j, ic = divmod(i, KC)
                nc.tensor.matmul(
                    u_ps[:],
                    w_slice(i, g),
                    x_chunks[j][:, ts(ic, TT)],
                    start=(i == 0),
                    stop=(i == KI - 1),
                )

            def emit_scan(t, g, u_ps):
                hb = hb_pool.tile([128, TT], BF16, tag="hb")
                init = 0.0 if t == 0 else hb_prev[g][:, TT - 1: TT]
                nc.vector.tensor_tensor_scan(
                    hb[:],
                    lam_sb[:, g:g + 1].to_broadcast((128, TT)),
                    u_ps[:],
                    init,
                    op0=mybir.AluOpType.mult,
                    op1=mybir.AluOpType.add,
                )
                hb_prev[g] = hb
                hb_tiles[t * NG + g] = hb
                if t == 0:
                    dma_c(g, hb)
                if t == NT - 1:
                    # local end state -> e (f32 upcast of the bf16 column)
                    nc.vector.tensor_copy(e_sb[:, g:g + 1], hb[:, TT - 1: TT])

            def emit_mm1_g(t, g, x_chunks):
                u_ps = u_psum.tile([128, TT], F32, tag="u")
                for i in range(KI):
                    mm1_one(u_ps, x_chunks, g, i)
                emit_scan(t, g, u_ps)

            def emit_mm1_t0(x_chunks):
                # pass A: one 8-wide i-major pass (g0-7, all 8 psum banks):
                # w_i consumption pace 1.7us/tile, safely behind the startup
                # DMA stream.  Then g8-15 as ordinary sequential chains (one
                # psum buf per 3.4us vs pass-A scans freeing one per 1.3us),
                # so the second half never stalls on psum recycling.
                u_list = [
                    (u_psum if q < 4 else y_psum).tile(
                        [128, TT], F32, tag="u" if q < 4 else "y",
                        name=f"u_t0_{q}")
                    for q in range(8)
                ]
                for i in range(KI):
                    for q in range(8):
                        mm1_one(u_list[q], x_chunks, q, i)
                for q in range(8):
                    emit_scan(0, q, u_list[q])
                for g in range(8, NG):
                    emit_mm1_g(0, g, x_chunks)

            def emit_mm2_o(t, o):
                y_ps = y_psum.tile([128, TT], F32, tag="y")
                for g in range(NG):
                    nc.tensor.matmul(
                        y_ps[:],
                        c_sb[g][:, ts(o, 128)],
                        hb_tiles[t * NG + g][:],
                        start=(g == 0),
                        stop=(g == NG - 1),
                    )
                wide = 2 if t >= NT - 1 else 4   # smaller stages shorten the drain tail
                oj, oc = divmod(o, wide)
                if oc == 0:
                    y_stage[0] = yst_pool.tile(
                        [128, wide * TT], BF16, tag=f"yst{wide}", name=f"yst_{t}_{oj}"
                    )
                eng = nc.vector.tensor_copy if (o % 2 == 0) else nc.scalar.copy
                eng(y_stage[0][:, ts(oc, TT)], y_ps[:])
                if oc == wide - 1:
                    # final slab: x stream is done -> alternate both queues so
                    # the last two stage DMAs don't serialize on one ring
                    if t >= NT - 1:
                        deng = nc.sync if (oj % 2 == 0) else nc.scalar
                        deng.dma_start(yo2[t, :, oj], y_stage[0][:])
                    else:
                        nc.scalar.dma_start(yo3[t, :, oj], y_stage[0][:])

            y_stage = [None]

            for t in range(NT):
                if t == 0:
                    emit_mm1_t0(x0_chunks)
                else:
                    x_chunks = dma_x(t)
                    # slab 1 lags mm2 by 2 chains so the slab-0 pass-B scans
                    # (which free the borrowed y-psum bufs) stay ahead
                    lag = 2 if t == 1 else 0
                    for g in range(NG):
                        emit_mm1_g(t, g, x_chunks)
                        if g >= lag:
                            emit_mm2_o(t - 1, g - lag)
                    for o in range(NO - lag, NO):
                        emit_mm2_o(t - 1, o)
            # e only depends on the slab-3 scans: emit BEFORE the mm2 tail so
            # it never queues behind the final y stage DMAs (its ~2.6us
            # transfer+receipt would otherwise extend the span end)
            nc.sync.dma_start(e[:], e_sb[:])
            for o in range(NO):
                emit_mm2_o(NT - 1, o)

    nc.compile()
    return nc


_NC_CACHE = None


def _get_nc():
    global _NC_CACHE
    if _NC_CACHE is None:
        _NC_CACHE = _build_nc()
    return _NC_CACHE


def _prep_in_maps(xs, lam, w_in, c_out):
    w_t = np.ascontiguousarray(w_in.T).astype(NP_BF16)        # [I, N]
    c_t = np.ascontiguousarray(c_out.T).astype(NP_BF16)       # [N, I]
    lamb = np.ascontiguousarray(
        lam.reshape(NG, 128).T.astype(np.float32)             # [128, NG]
    )
    in_maps = []
    for k in range(NCORES):
        xk = np.ascontiguousarray(xs[k * T:(k + 1) * T].T).astype(NP_BF16)  # [I, T]
        xt = (
            xk.reshape(KI, 128, NT, TT)
            .transpose(2, 1, 0, 3)
            .reshape(NT, 128, KI * TT)
        )
        in_maps.append({
            "xt": np.ascontiguousarray(xt),
            "wt": w_t,
            "ct": c_t,
            "lamb": lamb,
        })
    return in_maps


def _decay_basis(lam64, r=48):
    """Anchor-basis least-squares factorization of Lpow[n,t] = lam_n^(t+1),
    t = 0..T-1:  Lpow ~= Aco @ B.  Closed-form Gram matrices (geometric
    sums), so the fit costs ~ms.  Returns (Aco [N,r], B [r,T])."""
    tpow = np.arange(1, T + 1)
    lmax = float(lam64.max())
    eps_lo = max(1e-7, 1.0 - lmax)
    mus = 1.0 - np.logspace(np.log10(eps_lo), 0.0, r)
    mus = np.unique(np.clip(mus, 0.0, lmax))[::-1]

    def geo(a):  # sum_{t=1..T} a^t
        a = np.asarray(a, np.float64)
        return np.where(np.abs(1 - a) < 1e-12, float(T), a * (1 - a ** T) / (1 - a))

    G = geo(mus[:, None] * mus[None, :])
    R = geo(lam64[:, None] * mus[None, :])
    Aco = np.linalg.solve(G + 1e-9 * np.eye(len(mus)), R.T).T
    B = mus[:, None] ** tpow[None, :]
    return Aco, B


def combine_outputs(results, xs, lam, c_out, d_skip):
    """results: per-core {"y": [NT,128,NO*TT] bf16, "e": [128,NG] f32}
    -> full Y [L, I] f32 with host-side cross-chunk carry correction."""
    lam64 = lam.astype(np.float64)
    lamT = lam64 ** T
    Aco, B = _decay_basis(lam64)
    c64 = c_out.astype(np.float64)

    out = np.empty((L, I), dtype=np.float32)
    carry = np.zeros(N, dtype=np.float64)   # global state entering chunk k
    for k in range(NCORES):
        yk = results[k]["y"].astype(np.float32)
        yk = yk.reshape(NT, 128, NO, TT).transpose(2, 1, 0, 3).reshape(I, T)
        if k > 0:
            # H_k[t] = H_local_k[t] + lam^{t+1} * carry  ->  Y correction
            ycorr = (c64 @ (carry[:, None] * Aco)) @ B      # [I, T]
            yk = yk + ycorr.astype(np.float32)
        out[k * T:(k + 1) * T] = yk.T
        e_k = results[k]["e"].astype(np.float64).T.reshape(N)   # (p,g)->g*128+p
        # global end of chunk k seeds chunk k+1
        carry = lamT * carry + e_k
    return out + (xs * d_skip[None, :].astype(np.float32))


def run_on_hw(xs, lam, w_in, c_out, d_skip):
    nc = _get_nc()
    in_maps = _prep_in_maps(xs, lam, w_in, c_out)
    res = run_bass_kernel_spmd(nc, in_maps, core_ids=list(range(NCORES)))
    return combine_outputs(res.results, xs, lam, c_out, d_skip), res


def kernel(xs, lam, w_in, c_out, d_skip):
    out, _ = run_on_hw(
        np.asarray(xs, dtype=np.float32),
        np.asarray(lam, dtype=np.float32),
        np.asarray(w_in, dtype=np.float32),
        np.asarray(c_out, dtype=np.float32),
        np.asarray(d_skip, dtype=np.float32),
    )
    return out


# revision 8
# speedup vs baseline: 1.0030x; 1.0030x over previous
"""Trainium2 Bass kernel for BaseSSMLayer — sequence-sharded (v2).

Computation (verified equivalent to the reference's associative_scan):
    U = xs @ w_in.T              # [L, N]
    h_t = lam * h_{t-1} + U_t    # linear recurrence over L
    Y = H @ c_out.T + xs * d_skip

Strategy: data-parallel over the sequence (L=16384 -> 2048 steps/core,
8 cores).  Each core holds the FULL weights (w 8MB + c 8MB bf16) and
computes its chunk's mm1 -> local scan (zero init) -> mm2 exactly like
the v1 tensor-parallel kernel, but with 4x less HBM traffic per core
(32MB vs 130MB): no full-xs broadcast, no full-size partial-Y output.
Less DMA energy also means less GPIO/P0 clock throttling, which set the
worst-core span in v1.

The scan's cross-chunk carry is fixed up on the HOST: each core outputs
its local end-state e_k ([2048] f32, 8KB).  The host chains
carry_k = lam^T * carry_{k-1} + e_{k-1} and adds the correction
    Y_k += c_out @ (carry_k * lam^{t+1})
via a low-rank factorization of the decay-profile matrix
Lpow[n, t] = lam_n^{t+1}  (exponential profiles are numerically
low-rank; anchor-basis least squares with closed-form Gram matrices
gives rel err ~1e-5 at r=48).  No cross-core communication on device.
"""

import numpy as np
import ml_dtypes

import concourse.tile as tile
from concourse import bacc, mybir
from concourse.bass import ts
from concourse.bass_utils import run_bass_kernel_spmd

L = 16384        # sequence length
I = 2048         # in_dim (= out dim of Y)
N = 2048         # state_dim
NCORES = 8
T = L // NCORES          # 2048 timesteps per core
TT = 512                 # time-tile (free dim per matmul / scan)
NT = T // TT             # 4 time slabs per core
KI = I // 128            # 16 contraction tiles over in_dim (mm1)
NG = N // 128            # 16 state-channel groups (scan partitions, mm2 contraction)
NO = I // 128            # 16 output blocks (mm2)
NCH = 4                  # x DMA chunking: KI split into NCH chunks of KC i-tiles
KC = KI // NCH

BF16 = mybir.dt.bfloat16
F32 = mybir.dt.float32
NP_BF16 = ml_dtypes.bfloat16


def _build_nc():
    nc = bacc.Bacc(
        "TRN2",
        target_bir_lowering=False,
        debug=False,
        num_devices=NCORES,
    )
    xt = nc.dram_tensor("xt", [NT, 128, KI * TT], BF16, kind="ExternalInput").ap()
    wt = nc.dram_tensor("wt", [I, N], BF16, kind="ExternalInput").ap()
    ct = nc.dram_tensor("ct", [N, I], BF16, kind="ExternalInput").ap()
    lamb = nc.dram_tensor("lamb", [128, NG], F32, kind="ExternalInput").ap()
    y = nc.dram_tensor("y", [NT, 128, NO * TT], BF16, kind="ExternalOutput").ap()
    e = nc.dram_tensor("e", [128, NG], F32, kind="ExternalOutput").ap()

    with tile.TileContext(nc) as tc:
        with (
            tc.tile_pool(name="const", bufs=1) as const_pool,
            tc.tile_pool(name="xin", bufs=6) as x_pool,
            tc.tile_pool(name="hb", bufs=34) as hb_pool,
            tc.tile_pool(name="yst", bufs=3) as yst_pool,
            tc.tile_pool(name="ups", bufs=4, space="PSUM") as u_psum,
            tc.tile_pool(name="yps", bufs=4, space="PSUM") as y_psum,
        ):
            # --- startup DMA orchestration: w split across BOTH queues,
            # interleaved with the slab-0 x chunks in consumption order so
            # the slab-0 mm1 rounds never starve (stalls >3.4us would also
            # re-throttle the PE clock governor). ---
            lam_sb = const_pool.tile([128, NG], F32, tag="lam")
            nc.scalar.dma_start(lam_sb[:], lamb[:])
            w_sb = [None] * KI
            xt3 = xt.rearrange("t p (j c) -> t p j c", j=NCH)

            def dma_w(i, eng):
                w = const_pool.tile([128, N], BF16, tag=f"w{i}", name=f"w{i}")
                eng.dma_start(w[:], wt[i * 128:(i + 1) * 128, :])
                w_sb[i] = w

            def w_slice(i, g):
                return w_sb[i][:, g * 128:(g + 1) * 128]

            def dma_x_chunk(t, j, eng):
                xc = x_pool.tile([128, KC * TT], BF16, tag="x", name=f"x{t}_{j}")
                eng.dma_start(xc[:], xt3[t, :, j])
                return xc

            # startup: 512KB granules, both queues, ordered by first-need time
            #   scalar: lam w0 w2 w4 w6 w8 w10 w12 w14   (then idles; c gated)
            #   sync:   x0c0 w1 w3 x0c1 w5 w7 x0c2 w9 w11 x0c3 w13 w15
            x0_chunks = [None] * NCH
            x0_chunks[0] = dma_x_chunk(0, 0, nc.sync)
            dma_w(1, nc.sync)
            dma_w(3, nc.sync)
            x0_chunks[1] = dma_x_chunk(0, 1, nc.sync)
            dma_w(5, nc.sync)
            dma_w(7, nc.sync)
            x0_chunks[2] = dma_x_chunk(0, 2, nc.sync)
            dma_w(9, nc.sync)
            dma_w(11, nc.sync)
            x0_chunks[3] = dma_x_chunk(0, 3, nc.sync)
            dma_w(13, nc.sync)
            dma_w(15, nc.sync)
            for i in (0, 2, 4, 6, 8, 10, 12, 14):
                dma_w(i, nc.scalar)
            # Pre-warm the PE clock governor while the startup DMA lands
            # (512-wide moving operand keeps the governor at 2.4 GHz); the
            # slab-0 8-wide i-major passes only need w_0/x0c0 to start, so a
            # short bridge suffices.
            warm_sb = const_pool.tile([128, 512], BF16, tag="warm")
            nc.vector.memset(warm_sb[:], 0.0)
            warm_ps = u_psum.tile([128, TT], F32, tag="u")
            for _ in range(36):
                nc.tensor.matmul(warm_ps[:], warm_sb[:, :128], warm_sb[:], start=True, stop=True)

            c_sb = [None] * NG
            gate_sb = const_pool.tile([128, NG], F32, tag="gate")

            def dma_c(g, hb):
                # c loads ride the scalar queue but are gated behind a tiny
                # ACT copy that reads slab-0 scan g's output, so their 8MB
                # never contends with the critical w/x startup DMA.
                nc.scalar.copy(gate_sb[:, g:g + 1], hb[:, :1])
                c = const_pool.tile([128, I], BF16, tag=f"c{g}", name=f"c{g}")
                nc.scalar.dma_start(c[:], ct[g * 128:(g + 1) * 128, :])
                c_sb[g] = c

            e_sb = const_pool.tile([128, NG], F32, tag="e")

            hb_prev = [None] * NG          # bf16 H tile of previous slab, per group
            hb_tiles = [None] * (NT * NG)  # bf16 H tiles pending mm2
            yo3 = y.rearrange("t p (j c) -> t p j c", j=NO // 4)
            yo2 = y.rearrange("t p (j c) -> t p j c", j=NO // 2)

            def dma_x(t):
                return [dma_x_chunk(t, j, nc.sync) for j in range(NCH)]

            def mm1_one(u_ps, x_chunks, g, i):
                j, ic = divmod(i, KC)
                nc.tensor.matmul(
                    u_ps[:],
                    w_slice(i, g),
                    x_chunks[j][:, ts(ic, TT)],
                    start=(i == 0),
                    stop=(i == KI - 1),
                )

            def emit_scan(t, g, u_ps):
                hb = hb_pool.tile([128, TT], BF16, tag="hb")
                init = 0.0 if t == 0 else hb_prev[g][:, TT - 1: TT]
                nc.vector.tensor_tensor_scan(
                    hb[:],
                    lam_sb[:, g:g + 1].to_broadcast((128, TT)),
                    u_ps[:],
                    init,
                    op0=mybir.AluOpType.mult,
                    op1=mybir.AluOpType.add,
                )
                hb_prev[g] = hb
                hb_tiles[t * NG + g] = hb
                if t == 0:
                    dma_c(g, hb)
                if t == NT - 1:
                    # local end state -> e (f32 upcast of the bf16 column)
                    nc.vector.tensor_copy(e_sb[:, g:g + 1], hb[:, TT - 1: TT])

            def emit_mm1_g(t, g, x_chunks):
                u_ps = u_psum.tile([128, TT], F32, tag="u")
                for i in range(KI):
                    mm1_one(u_ps, x_chunks, g, i)
                emit_scan(t, g, u_ps)

            def emit_mm1_t0(x_chunks):
                # two 8-wide i-major passes (8 concurrent psum chains across
                # both pools = all 8 banks): w_i consumption pace is 1.7us per
                # tile, safely behind the startup DMA stream.  (A sequential-
                # chain variant for g8-15 measured ~2us faster medians but a
                # wider worst-core tail — the 8-wide passes amortize per-core
                # DMA jitter, and the grade is the max core span.)
                for r in range(2):
                    u_list = [
                        (u_psum if q < 4 else y_psum).tile(
                            [128, TT], F32, tag="u" if q < 4 else "y",
                            name=f"u_t0_{r}_{q}")
                        for q in range(8)
                    ]
                    for i in range(KI):
                        for q in range(8):
                            mm1_one(u_list[q], x_chunks, 8 * r + q, i)
                    for q in range(8):
                        emit_scan(0, 8 * r + q, u_list[q])

            def emit_mm2_o(t, o):
                y_ps = y_psum.tile([128, TT], F32, tag="y")
                for g in range(NG):
                    nc.tensor.matmul(
                        y_ps[:],
                        c_sb[g][:, ts(o, 128)],
                        hb_tiles[t * NG + g][:],
                        start=(g == 0),
                        stop=(g == NG - 1),
                    )
                wide = 2 if t >= NT - 1 else 4   # smaller stages shorten the drain tail
                oj, oc = divmod(o, wide)
                if oc == 0:
                    y_stage[0] = yst_pool.tile(
                        [128, wide * TT], BF16, tag=f"yst{wide}", name=f"yst_{t}_{oj}"
                    )
                eng = nc.vector.tensor_copy if (o % 2 == 0) else nc.scalar.copy
                eng(y_stage[0][:, ts(oc, TT)], y_ps[:])
                if oc == wide - 1:
                    # final slab: x stream is done -> alternate both queues so
                    # the last two stage DMAs don't serialize on one ring
                    if t >= NT - 1:
                        deng = nc.sync if (oj % 2 == 0) else nc.scalar
                        deng.dma_start(yo2[t, :, oj], y_stage[0][:])
                    else:
                        nc.scalar.dma_start(yo3[t, :, oj], y_stage[0][:])

            y_stage = [None]

            for t in range(NT):
                if t == 0:
                    emit_mm1_t0(x0_chunks)
                else:
                    x_chunks = dma_x(t)
                    # slab 1 lags mm2 by 2 chains so the slab-0 pass-B scans
                    # (which free the borrowed y-psum bufs) stay ahead
                    lag = 2 if t == 1 else 0
                    for g in range(NG):
                        emit_mm1_g(t, g, x_chunks)
                        if g >= lag:
                            emit_mm2_o(t - 1, g - lag)
                    for o in range(NO - lag, NO):
                        emit_mm2_o(t - 1, o)
            # e only depends on the slab-3 scans: emit BEFORE the mm2 tail so
            # it never queues behind the final y stage DMAs (its ~2.6us
            # transfer+receipt would otherwise extend the span end)
            nc.sync.dma_start(e[:], e_sb[:])
            for o in range(NO):
                emit_mm2_o(NT - 1, o)

    nc.compile()
    return nc


_NC_CACHE = None


def _get_nc():
    global _NC_CACHE
    if _NC_CACHE is None:
        _NC_CACHE = _build_nc()
    return _NC_CACHE


def _prep_in_maps(xs, lam, w_in, c_out):
    w_t = np.ascontiguousarray(w_in.T).astype(NP_BF16)        # [I, N]
    c_t = np.ascontiguousarray(c_out.T).astype(NP_BF16)       # [N, I]
    lamb = np.ascontiguousarray(
        lam.reshape(NG, 128).T.astype(np.float32)             # [128, NG]
    )
    in_maps = []
    for k in range(NCORES):
        xk = np.ascontiguousarray(xs[k * T:(k + 1) * T].T).astype(NP_BF16)  # [I, T]
        xt = (
            xk.reshape(KI, 128, NT, TT)
            .transpose(2, 1, 0, 3)
            .reshape(NT, 128, KI * TT)
        )
        in_maps.append({
            "xt": np.ascontiguousarray(xt),
            "wt": w_t,
            "ct": c_t,
            "lamb": lamb,
        })
    return in_maps


def _decay_basis(lam64, r=48):
    """Anchor-basis least-squares factorization of Lpow[n,t] = lam_n^(t+1),
    t = 0..T-1:  Lpow ~= Aco @ B.  Closed-form Gram matrices (geometric
    sums), so the fit costs ~ms.  Returns (Aco [N,r], B [r,T])."""
    tpow = np.arange(1, T + 1)
    lmax = float(lam64.max())
    eps_lo = max(1e-7, 1.0 - lmax)
    mus = 1.0 - np.logspace(np.log10(eps_lo), 0.0, r)
    mus = np.unique(np.clip(mus, 0.0, lmax))[::-1]

    def geo(a):  # sum_{t=1..T} a^t
        a = np.asarray(a, np.float64)
        return np.where(np.abs(1 - a) < 1e-12, float(T), a * (1 - a ** T) / (1 - a))

    G = geo(mus[:, None] * mus[None, :])
    R = geo(lam64[:, None] * mus[None, :])
    Aco = np.linalg.solve(G + 1e-9 * np.eye(len(mus)), R.T).T
    B = mus[:, None] ** tpow[None, :]
    return Aco, B


def combine_outputs(results, xs, lam, c_out, d_skip):
    """results: per-core {"y": [NT,128,NO*TT] bf16, "e": [128,NG] f32}
    -> full Y [L, I] f32 with host-side cross-chunk carry correction."""
    lam64 = lam.astype(np.float64)
    lamT = lam64 ** T
    Aco, B = _decay_basis(lam64)
    c64 = c_out.astype(np.float64)

    out = np.empty((L, I), dtype=np.float32)
    carry = np.zeros(N, dtype=np.float64)   # global state entering chunk k
    for k in range(NCORES):
        yk = results[k]["y"].astype(np.float32)
        yk = yk.reshape(NT, 128, NO, TT).transpose(2, 1, 0, 3).reshape(I, T)
        if k > 0:
            # H_k[t] = H_local_k[t] + lam^{t+1} * carry  ->  Y correction
            ycorr = (c64 @ (carry[:, None] * Aco)) @ B      # [I, T]
            yk = yk + ycorr.astype(np.float32)
        out[k * T:(k + 1) * T] = yk.T
        e_k = results[k]["e"].astype(np.float64).T.reshape(N)   # (p,g)->g*128+p
        # global end of chunk k seeds chunk k+1
        carry = lamT * carry + e_k
    return out + (xs * d_skip[None, :].astype(np.float32))


def run_on_hw(xs, lam, w_in, c_out, d_skip):
    nc = _get_nc()
    in_maps = _prep_in_maps(xs, lam, w_in, c_out)
    res = run_bass_kernel_spmd(nc, in_maps, core_ids=list(range(NCORES)))
    return combine_outputs(res.results, xs, lam, c_out, d_skip), res


def kernel(xs, lam, w_in, c_out, d_skip):
    out, _ = run_on_hw(
        np.asarray(xs, dtype=np.float32),
        np.asarray(lam, dtype=np.float32),
        np.asarray(w_in, dtype=np.float32),
        np.asarray(c_out, dtype=np.float32),
        np.asarray(d_skip, dtype=np.float32),
    )
    return out
